# revision 9
# baseline (speedup 1.0000x reference)
"""CompressedLinear (int8 weight, per-row scale) on 8 Trainium2 NeuronCores.

Math: y[b,s,o] = sum_i x[b,s,i] * (w_int8[o,i] * scale[o]) + bias[o]

Strategy (tensor-parallel over out_features, per sharding hint):
  - Shard W/scale/bias rows across 8 cores (1376 rows each); x replicated.
  - Scale is applied to the matmul OUTPUT (algebraically identical), so the
    device matmuls run on the raw int8 weights cast to fp16 (int8 is exact
    in fp16).
  - Single fp16 matmul pass: casting x to fp16 (on the HOST, which halves
    the x wire traffic) bounds the output relative error at ~2e-4.
  - Each core computes yT[o_shard, s] = W_shard @ x^T; the host hands each
    core pre-transposed views: xt = x^T [4096, 2048] fp16 and
    wt = W_shard^T [4096, 1376] int8.
  - DMA *instruction issue* costs ~0.7-1.0us per dma_start, so loads are
    batched into multi-k-slice blocks (3D access patterns). In-flight DMA
    descriptors round-robin across the engine pool, so an unchained flood
    makes the FIRST block finish last; the w stream is serialized
    (block i waits on block i-1's completion) and chunk-0 x is chained at
    depth 2 with small head blocks, so the first (x, w) pair lands ~9us
    and the PE never starves nor re-throttles after warm-up.
  - Per-partition affine (scale, bias) is fused into the PSUM eviction.
  - The very last PSUM group runs kt-inner per o-tile, and the final
    o-tile is split into two s-halves, so the last evictions/output DMAs
    stagger into the matmul stream instead of serializing at the end.
"""

import os
import numpy as np

import concourse.bass as bass
import concourse.tile as tile
from concourse import bacc, mybir
from concourse.bass_utils import run_bass_kernel_spmd

B = 1
S = 2048
I = 4096
O = 11008
N_CORES = 8
O_SHARD = O // N_CORES  # 1376
S_CHUNK = 512
P = 128
KB = 8    # k-slices per x block, chunks 1+
X0_BLOCKS = [2, 2, 4, 4, 4, 4, 4, 4, 4]  # k-slices per chunk-0 block


def build_bass(I_=I, O_SHARD_=O_SHARD, S_=S, S_CHUNK_=S_CHUNK):
    KT = I_ // P
    N_CHUNKS = S_ // S_CHUNK_
    OT = (O_SHARD_ + P - 1) // P

    MM_DT = mybir.dt.float16
    nc = bacc.Bacc("TRN2", target_bir_lowering=False, debug=False)

    xt = nc.dram_tensor("xt", [I_, S_], mybir.dt.float16, kind="ExternalInput").ap()
    wt = nc.dram_tensor("wt", [I_, O_SHARD_], mybir.dt.int8, kind="ExternalInput").ap()
    scale = nc.dram_tensor("scale", [O_SHARD_], mybir.dt.float32, kind="ExternalInput").ap()
    bias = nc.dram_tensor("bias", [O_SHARD_], mybir.dt.float32, kind="ExternalInput").ap()
    yt = nc.dram_tensor("yt", [O_SHARD_, S_], mybir.dt.float32, kind="ExternalOutput").ap()

    # PSUM bank groups: 4+4+3 o-tiles so two adjacent groups fit in the
    # 8 banks and group transitions never wait on drains.
    groups = []
    g0 = 0
    for gsz in (4, 4, 3):
        if g0 < OT:
            groups.append((g0, min(g0 + gsz, OT)))
            g0 += gsz
    full_t = O_SHARD_ // P
    rem = O_SHARD_ - full_t * P

    with tile.TileContext(nc) as tc:
        with (
            tc.tile_pool(name="wres", bufs=1) as wres_pool,
            tc.tile_pool(name="consts", bufs=1) as const_pool,
            tc.tile_pool(name="xc0", bufs=1) as x0_pool,
            tc.tile_pool(name="xcn", bufs=2 * (KT // KB)) as xn_pool,
            tc.tile_pool(name="outp", bufs=4) as out_pool,
            tc.tile_pool(name="psum", bufs=8, space="PSUM") as psum_pool,
        ):
            # PE warm-up: dependency-free matmuls on a zeroed tile keep the
            # PE busy from right after the preamble, so the HAM clock gate
            # opens (K=8/8) around when the first real matmuls flow.
            warm_sb = const_pool.tile([P, P], MM_DT)
            nc.any.memset(warm_sb[:], 0.0)
            warm_ps = psum_pool.tile([P, P], mybir.dt.float32, name="warm_ps", tag="psum")
            N_WARM = 16
            for i in range(N_WARM):
                nc.tensor.matmul(
                    warm_ps[:], warm_sb[:], warm_sb[:],
                    start=(i == 0), stop=(i == N_WARM - 1),
                )

            def x_dma(pool, kt0, kb, s0, tag, queue):
                bt = pool.tile([P, kb * S_CHUNK_], MM_DT, tag=tag)
                src = xt[kt0 * P:(kt0 + kb) * P, s0:s0 + S_CHUNK_]
                dd = queue.dma_start(
                    bt[:].rearrange("p (kt s) -> p kt s", s=S_CHUNK_),
                    src.rearrange("(kt p) s -> p kt s", p=P))
                return bt, dd

            # chunk-0 x: small head blocks, chained at depth 2 so at most
            # two descriptors are in flight (an unchained flood would
            # round-robin and delay the first block's completion).
            def emit_x_chunk0():
                blocks = []  # (kt0, kb, tile)
                dds = []
                kt0 = 0
                for i, kb in enumerate(X0_BLOCKS):
                    bt, dd = x_dma(x0_pool, kt0, kb, 0, f"x0b{i}", nc.sync)
                    if i >= 2:
                        bass._add_dep_helper(
                            dd.ins, dds[i - 2].ins, sync=True,
                            reason="depth-2 chain: keep first x blocks low-latency",
                        )
                    blocks.append((kt0, kb, bt))
                    dds.append(dd)
                    kt0 += kb
                def rhs(kt, blocks=blocks):
                    for kt0, kb, bt in blocks:
                        if kt0 <= kt < kt0 + kb:
                            return bt[:, (kt - kt0) * S_CHUNK_:(kt - kt0 + 1) * S_CHUNK_]
                    raise KeyError(kt)
                return rhs

            def emit_x_chunk(sc):
                s0 = sc * S_CHUNK_
                blocks = []
                for b in range(KT // KB):
                    bt, _ = x_dma(xn_pool, b * KB, KB, s0, f"xb{KB}", nc.sync)
                    blocks.append(bt)
                def rhs(kt, blocks=blocks):
                    return blocks[kt // KB][:, (kt % KB) * S_CHUNK_:(kt % KB + 1) * S_CHUNK_]
                return rhs

            # per-partition scale/bias columns on the scalar queue (tiny;
            # must not delay the x stream on sync).
            scale_t = const_pool.tile([P, OT], mybir.dt.float32)
            bias_t = const_pool.tile([P, OT], mybir.dt.float32)
            if full_t:
                nc.scalar.dma_start(
                    scale_t[:, :full_t], scale[: full_t * P].rearrange("(t p) -> p t", p=P)
                )
                nc.scalar.dma_start(
                    bias_t[:, :full_t], bias[: full_t * P].rearrange("(t p) -> p t", p=P)
                )
            if rem:
                nc.scalar.dma_start(
                    scale_t[:rem, full_t:], scale[full_t * P:].rearrange("(t p) -> p t", p=rem)
                )
                nc.scalar.dma_start(
                    bias_t[:rem, full_t:], bias[full_t * P:].rearrange("(t p) -> p t", p=rem)
                )

            rhs0 = emit_x_chunk0()

            # Weight shard int8 -> fp16 (exact), kept resident in SBUF.
            # Casting DMAs can only run on the gpsimd (SWDGE) queue; batch
            # KWB k-slices per DMA and stream the three o-groups in
            # PSUM-sweep order, kt-ordered, fully serialized so each block
            # completes ~1.5-2us after the previous (first block gates the
            # first real matmul).
            KWB = 4
            w_blocks = [None] * len(groups)
            w_widths = [None] * len(groups)
            prev_wd = None
            for g, (g_start, g_end) in enumerate(groups):
                c0 = g_start * P
                c1 = min(g_end * P, O_SHARD_)
                wid = c1 - c0
                w_widths[g] = wid
                blocks = []
                for b in range(KT // KWB):
                    w_b = wres_pool.tile([P, KWB * wid], MM_DT, tag=f"wb{g}_{b}")
                    src = wt[b * KWB * P:(b + 1) * KWB * P, c0:c1]
                    wd = nc.gpsimd.dma_start(
                        w_b[:].rearrange("p (kt o) -> p kt o", o=wid),
                        src.rearrange("(kt p) o -> p kt o", p=P))
                    if prev_wd is not None:
                        bass._add_dep_helper(
                            wd.ins, prev_wd.ins, sync=True,
                            reason="serialize w stream: first block gates first matmul",
                        )
                    prev_wd = wd
                    blocks.append(w_b)
                w_blocks[g] = blocks

            def w_slice_for(kt, g, ot_local, orows):
                wid = w_widths[g]
                base = (kt % KWB) * wid + ot_local * P
                return w_blocks[g][kt // KWB][:, base:base + orows]

            def evict(sc, ot, psum_t, s_off=0, s_len=None):
                s_len = S_CHUNK_ if s_len is None else s_len
                s0 = sc * S_CHUNK_ + s_off
                orows = min(P, O_SHARD_ - ot * P)
                out_t = out_pool.tile([P, S_CHUNK_], mybir.dt.float32)
                nc.vector.tensor_scalar(
                    out=out_t[:orows, :s_len],
                    in0=psum_t[:orows, :s_len],
                    scalar1=scale_t[:orows, ot:ot + 1],
                    scalar2=bias_t[:orows, ot:ot + 1],
                    op0=mybir.AluOpType.mult,
                    op1=mybir.AluOpType.add,
                )
                nc.sync.dma_start(
                    yt[ot * P:ot * P + orows, s0:s0 + s_len],
                    out_t[:orows, :s_len],
                )

            def emit_groups(sc, rhs, tail=False):
                # kt outer / o-tile inner: each x block's last reader comes
                # early in the group sweep, so next-chunk loads spread over
                # the whole chunk instead of bunching at its tail.
                for g, (g_start, g_end) in enumerate(groups):
                    last_group = tail and g == len(groups) - 1
                    if last_group:
                        # kt-inner per o-tile: each o-tile completes ~7us
                        # apart, so evictions/output DMAs overlap the
                        # remaining matmuls. The final o-tile is further
                        # split into two s-halves for the same reason.
                        for ot in range(g_start, g_end):
                            orows = min(P, O_SHARD_ - ot * P)
                            halves = ((0, S_CHUNK_),) if ot < g_end - 1 else (
                                (0, S_CHUNK_ // 2), (S_CHUNK_ // 2, S_CHUNK_ // 2))
                            for s_off, s_len in halves:
                                ps = psum_pool.tile(
                                    [P, s_len], mybir.dt.float32,
                                    name=f"psum_{sc}_{ot}_{s_off}", tag="psum",
                                )
                                for kt in range(KT):
                                    w_slice = w_slice_for(kt, g, ot - g_start, orows)
                                    nc.tensor.matmul(
                                        ps[:orows, :], w_slice,
                                        rhs(kt)[:, s_off:s_off + s_len],
                                        start=(kt == 0), stop=(kt == KT - 1),
                                    )
                                evict(sc, ot, ps, s_off, s_len)
                        continue
                    psums = {}
                    for ot in range(g_start, g_end):
                        psums[ot] = psum_pool.tile(
                            [P, S_CHUNK_], mybir.dt.float32,
                            name=f"psum_{sc}_{ot}", tag="psum",
                        )
                    for kt in range(KT):
                        for ot in range(g_start, g_end):
                            orows = min(P, O_SHARD_ - ot * P)
                            w_slice = w_slice_for(kt, g, ot - g_start, orows)
                            nc.tensor.matmul(
                                psums[ot][:orows, :], w_slice, rhs(kt),
                                start=(kt == 0), stop=(kt == KT - 1),
                            )
                    for ot in range(g_start, g_end):
                        evict(sc, ot, psums[ot])

            # Software-pipelined emission: loads for chunk sc+1 are emitted
            # before chunk sc's matmul groups, so in the per-queue FIFO
            # streams next-chunk loads sit ahead of this chunk's PSUM
            # drains.
            prev = rhs0
            for sc in range(N_CHUNKS):
                if sc + 1 < N_CHUNKS:
                    nxt = emit_x_chunk(sc + 1)
                else:
                    nxt = None
                emit_groups(sc, prev, tail=(sc == N_CHUNKS - 1))
                prev = nxt

    nc.compile()
    return nc


_NC_CACHE = None


def _get_nc():
    global _NC_CACHE
    if _NC_CACHE is None:
        _NC_CACHE = build_bass()
    return _NC_CACHE


def run(inputs, trace=False, trace_cores=None, tmpdir=None):
    x = np.asarray(inputs["x"])
    w = np.asarray(inputs["weight_int8"])
    scale = np.asarray(inputs["scale"], dtype=np.float32)
    bias = np.asarray(inputs["bias"], dtype=np.float32)

    if w.dtype != np.int8:
        w = w.astype(np.int8)
    x2d = np.ascontiguousarray(x.reshape(S, I).astype(np.float32, copy=False))
    xtr = np.ascontiguousarray(x2d.T.astype(np.float16))  # [I, S] fp16

    in_maps = []
    for c in range(N_CORES):
        sl = slice(c * O_SHARD, (c + 1) * O_SHARD)
        in_maps.append({
            "xt": xtr,
            "wt": np.ascontiguousarray(w[sl, :].T),  # [I, O_SHARD]
            "scale": np.ascontiguousarray(scale[sl]),
            "bias": np.ascontiguousarray(bias[sl]),
        })

    nc = _get_nc()
    kwargs = {}
    if trace:
        kwargs["trace"] = True
        if trace_cores is not None:
            kwargs["trace_cores"] = trace_cores
        if tmpdir is not None:
            kwargs["tmpdir"] = tmpdir
    res = run_bass_kernel_spmd(nc, in_maps, core_ids=list(range(N_CORES)), **kwargs)

    yt_full = np.concatenate([res.results[c]["yt"] for c in range(N_CORES)], axis=0)
    out = np.ascontiguousarray(yt_full.T).reshape(B, S, O).astype(np.float32, copy=False)
    if trace:
        return out, res
    return out


def kernel(**inputs) -> np.ndarray:
    return run(inputs, trace=False)


# revision 10
# speedup vs baseline: 1.0185x; 1.0185x over previous
"""CompressedLinear (int8 weight, per-row scale) on 8 Trainium2 NeuronCores.

Math: y[b,s,o] = sum_i x[b,s,i] * (w_int8[o,i] * scale[o]) + bias[o]

Strategy (tensor-parallel over out_features, per sharding hint):
  - Shard W/scale/bias rows across 8 cores (1376 rows each); x replicated.
  - Scale is applied to the matmul OUTPUT (algebraically identical), so the
    device matmuls run on the raw int8 weights in fp16 (int8 is exact in
    fp16). A single fp16 pass with host-side fp16 x bounds the output
    relative error at ~2e-4.
  - Each core computes yT[o_shard, s] = W_shard @ x^T.
  - Load scheduling is latency-driven: a dma_start costs ~0.7-1us of queue
    issue time, in-flight descriptors round-robin across the DMA engines
    (a flood delays the first completion), and the gpsimd SWDGE *casting*
    path has ~6us fixed latency per block. So:
      * x chunk 0 rides the sync ring as multi-k-slice blocks with small
        head blocks, chained at depth 2 (low latency, full stream rate);
      * the first PSUM group's weights (o-columns 0:512) are pre-cast to
        fp16 on the host and ride the otherwise-idle scalar ring the same
        way, so the first (x, w) pair lands ~2us after the ~6.5us queue
        preamble and the PE never starves during the first group sweep;
      * groups 1/2 keep the wire-efficient int8->fp16 SWDGE cast stream
        (their deadlines are 30+us out), chained at depth 4.
  - Per-partition affine (scale, bias) is fused into the PSUM eviction.
  - The very last PSUM group runs kt-inner per o-tile, and the final
    o-tile is split into two s-halves, so the last evictions/output DMAs
    stagger into the matmul stream instead of serializing at the end.
"""

import os
import numpy as np

import concourse.bass as bass
import concourse.tile as tile
from concourse import bacc, mybir
from concourse.bass_utils import run_bass_kernel_spmd

B = 1
S = 2048
I = 4096
O = 11008
N_CORES = 8
O_SHARD = O // N_CORES  # 1376
S_CHUNK = 512
P = 128
KB = 8    # k-slices per x block, chunks 1+
HEAD_BLOCKS = [2, 2, 4, 4, 4, 4, 4, 4, 4]  # k-slices per block, chunk-0 x and g0 w
G0_W = 512  # o-columns in PSUM group 0 (4 o-tiles)


def build_bass(I_=I, O_SHARD_=O_SHARD, S_=S, S_CHUNK_=S_CHUNK):
    KT = I_ // P
    N_CHUNKS = S_ // S_CHUNK_
    OT = (O_SHARD_ + P - 1) // P

    MM_DT = mybir.dt.float16
    nc = bacc.Bacc("TRN2", target_bir_lowering=False, debug=False)

    xt = nc.dram_tensor("xt", [I_, S_], mybir.dt.float16, kind="ExternalInput").ap()
    # group-0 weights pre-cast to fp16 and pre-blocked on the host:
    # wg0[p, kt*512 + o] = W_shard^T[kt*128 + p, o] for o in [0, 512)
    wg0 = nc.dram_tensor("wg0", [P, KT * G0_W], mybir.dt.float16, kind="ExternalInput").ap()
    wt = nc.dram_tensor("wt", [I_, O_SHARD_], mybir.dt.int8, kind="ExternalInput").ap()
    scale = nc.dram_tensor("scale", [O_SHARD_], mybir.dt.float32, kind="ExternalInput").ap()
    bias = nc.dram_tensor("bias", [O_SHARD_], mybir.dt.float32, kind="ExternalInput").ap()
    yt = nc.dram_tensor("yt", [O_SHARD_, S_], mybir.dt.float32, kind="ExternalOutput").ap()

    # PSUM bank groups: 4+4+3 o-tiles so two adjacent groups fit in the
    # 8 banks and group transitions never wait on drains.
    groups = []
    g0 = 0
    for gsz in (4, 4, 3):
        if g0 < OT:
            groups.append((g0, min(g0 + gsz, OT)))
            g0 += gsz
    assert groups[0][1] * P == G0_W
    full_t = O_SHARD_ // P
    rem = O_SHARD_ - full_t * P

    with tile.TileContext(nc) as tc:
        with (
            tc.tile_pool(name="wres", bufs=1) as wres_pool,
            tc.tile_pool(name="consts", bufs=1) as const_pool,
            tc.tile_pool(name="xc0", bufs=1) as x0_pool,
            tc.tile_pool(name="xcn", bufs=2 * (KT // KB)) as xn_pool,
            tc.tile_pool(name="outp", bufs=4) as out_pool,
            tc.tile_pool(name="psum", bufs=8, space="PSUM") as psum_pool,
        ):
            # PE warm-up: dependency-free matmuls on a zeroed tile keep the
            # PE busy from right after the preamble, so the HAM clock gate
            # opens (K=8/8) around when the first real matmuls flow.
            warm_sb = const_pool.tile([P, P], MM_DT)
            nc.any.memset(warm_sb[:], 0.0)
            warm_ps = psum_pool.tile([P, P], mybir.dt.float32, name="warm_ps", tag="psum")
            N_WARM = 16
            for i in range(N_WARM):
                nc.tensor.matmul(
                    warm_ps[:], warm_sb[:], warm_sb[:],
                    start=(i == 0), stop=(i == N_WARM - 1),
                )

            def chain(dd, dds, depth, reason):
                if len(dds) >= depth:
                    bass._add_dep_helper(
                        dd.ins, dds[-depth].ins, sync=True, reason=reason)
                dds.append(dd)

            # chunk-0 x: small head blocks on sync, chained at depth 2 so
            # at most two descriptors are in flight (an unchained flood
            # round-robins and delays the first completion).
            def emit_x_chunk0():
                blocks = []  # (kt0, kb, tile)
                dds = []
                kt0 = 0
                for i, kb in enumerate(HEAD_BLOCKS):
                    bt = x0_pool.tile([P, kb * S_CHUNK_], MM_DT, tag=f"x0b{i}")
                    src = xt[kt0 * P:(kt0 + kb) * P, 0:S_CHUNK_]
                    dd = nc.sync.dma_start(
                        bt[:].rearrange("p (kt s) -> p kt s", s=S_CHUNK_),
                        src.rearrange("(kt p) s -> p kt s", p=P))
                    chain(dd, dds, 2, "depth-2 chain: low-latency x head")
                    blocks.append((kt0, kb, bt))
                    kt0 += kb
                def rhs(kt, blocks=blocks):
                    for kt0, kb, bt in blocks:
                        if kt0 <= kt < kt0 + kb:
                            return bt[:, (kt - kt0) * S_CHUNK_:(kt - kt0 + 1) * S_CHUNK_]
                    raise KeyError(kt)
                return rhs

            def emit_x_chunk(sc):
                s0 = sc * S_CHUNK_
                blocks = []
                for b in range(KT // KB):
                    bt = xn_pool.tile([P, KB * S_CHUNK_], MM_DT, tag=f"xb{KB}")
                    src = xt[b * KB * P:(b + 1) * KB * P, s0:s0 + S_CHUNK_]
                    nc.sync.dma_start(
                        bt[:].rearrange("p (kt s) -> p kt s", s=S_CHUNK_),
                        src.rearrange("(kt p) s -> p kt s", p=P))
                    blocks.append(bt)
                def rhs(kt, blocks=blocks):
                    return blocks[kt // KB][:, (kt % KB) * S_CHUNK_:(kt % KB + 1) * S_CHUNK_]
                return rhs

            rhs0 = emit_x_chunk0()

            # group-0 weights: host-pre-cast fp16 on the scalar ring,
            # same head-block/chaining scheme as x chunk 0. (The SWDGE
            # cast path has ~6us fixed latency - too slow for group 0.)
            wg0_blocks = []  # (kt0, kb, tile)
            dds = []
            kt0 = 0
            for i, kb in enumerate(HEAD_BLOCKS):
                w_b = wres_pool.tile([P, kb * G0_W], MM_DT, tag=f"wg0_{i}")
                dd = nc.scalar.dma_start(
                    w_b[:], wg0[:, kt0 * G0_W:(kt0 + kb) * G0_W])
                chain(dd, dds, 2, "depth-2 chain: low-latency w g0 head")
                wg0_blocks.append((kt0, kb, w_b))
                kt0 += kb

            # per-partition scale/bias columns, scalar ring after the g0
            # weights (tiny; first needed at the first eviction ~35us in).
            scale_t = const_pool.tile([P, OT], mybir.dt.float32)
            bias_t = const_pool.tile([P, OT], mybir.dt.float32)
            if full_t:
                nc.scalar.dma_start(
                    scale_t[:, :full_t], scale[: full_t * P].rearrange("(t p) -> p t", p=P)
                )
                nc.scalar.dma_start(
                    bias_t[:, :full_t], bias[: full_t * P].rearrange("(t p) -> p t", p=P)
                )
            if rem:
                nc.scalar.dma_start(
                    scale_t[:rem, full_t:], scale[full_t * P:].rearrange("(t p) -> p t", p=rem)
                )
                nc.scalar.dma_start(
                    bias_t[:rem, full_t:], bias[full_t * P:].rearrange("(t p) -> p t", p=rem)
                )

            # groups 1/2: int8 -> fp16 SWDGE cast stream on gpsimd (wire-
            # efficient; deadlines 30+us out), KWB k-slices per block,
            # chained at depth 4 to balance latency and aggregate rate.
            KWB = 4
            w_blocks = [None] * len(groups)
            w_widths = [None] * len(groups)
            dds = []
            for g, (g_start, g_end) in enumerate(groups):
                if g == 0:
                    continue
                c0 = g_start * P
                c1 = min(g_end * P, O_SHARD_)
                wid = c1 - c0
                w_widths[g] = wid
                blocks = []
                for b in range(KT // KWB):
                    w_b = wres_pool.tile([P, KWB * wid], MM_DT, tag=f"wb{g}_{b}")
                    src = wt[b * KWB * P:(b + 1) * KWB * P, c0:c1]
                    dd = nc.gpsimd.dma_start(
                        w_b[:].rearrange("p (kt o) -> p kt o", o=wid),
                        src.rearrange("(kt p) o -> p kt o", p=P))
                    chain(dd, dds, 4, "depth-4 chain: pace SWDGE w cast stream")
                    blocks.append(w_b)
                w_blocks[g] = blocks

            def w_slice_for(kt, g, ot_local, orows):
                if g == 0:
                    for kt0, kb, w_b in wg0_blocks:
                        if kt0 <= kt < kt0 + kb:
                            base = (kt - kt0) * G0_W + ot_local * P
                            return w_b[:, base:base + orows]
                    raise KeyError(kt)
                wid = w_widths[g]
                base = (kt % KWB) * wid + ot_local * P
                return w_blocks[g][kt // KWB][:, base:base + orows]

            def evict(sc, ot, psum_t, s_off=0, s_len=None):
                s_len = S_CHUNK_ if s_len is None else s_len
                s0 = sc * S_CHUNK_ + s_off
                orows = min(P, O_SHARD_ - ot * P)
                out_t = out_pool.tile([P, S_CHUNK_], mybir.dt.float32)
                nc.vector.tensor_scalar(
                    out=out_t[:orows, :s_len],
                    in0=psum_t[:orows, :s_len],
                    scalar1=scale_t[:orows, ot:ot + 1],
                    scalar2=bias_t[:orows, ot:ot + 1],
                    op0=mybir.AluOpType.mult,
                    op1=mybir.AluOpType.add,
                )
                nc.sync.dma_start(
                    yt[ot * P:ot * P + orows, s0:s0 + s_len],
                    out_t[:orows, :s_len],
                )

            def emit_groups(sc, rhs, tail=False):
                # kt outer / o-tile inner: each x block's last reader comes
                # early in the group sweep, so next-chunk loads spread over
                # the whole chunk instead of bunching at its tail.
                for g, (g_start, g_end) in enumerate(groups):
                    last_group = tail and g == len(groups) - 1
                    if last_group:
                        # kt-inner per o-tile: each o-tile completes ~7us
                        # apart, so evictions/output DMAs overlap the
                        # remaining matmuls. The final o-tile is further
                        # split into two s-halves for the same reason.
                        for ot in range(g_start, g_end):
                            orows = min(P, O_SHARD_ - ot * P)
                            halves = ((0, S_CHUNK_),) if ot < g_end - 1 else (
                                (0, S_CHUNK_ // 2), (S_CHUNK_ // 2, S_CHUNK_ // 2))
                            for s_off, s_len in halves:
                                ps = psum_pool.tile(
                                    [P, s_len], mybir.dt.float32,
                                    name=f"psum_{sc}_{ot}_{s_off}", tag="psum",
                                )
                                for kt in range(KT):
                                    w_slice = w_slice_for(kt, g, ot - g_start, orows)
                                    nc.tensor.matmul(
                                        ps[:orows, :], w_slice,
                                        rhs(kt)[:, s_off:s_off + s_len],
                                        start=(kt == 0), stop=(kt == KT - 1),
                                    )
                                evict(sc, ot, ps, s_off, s_len)
                        continue
                    psums = {}
                    for ot in range(g_start, g_end):
                        psums[ot] = psum_pool.tile(
                            [P, S_CHUNK_], mybir.dt.float32,
                            name=f"psum_{sc}_{ot}", tag="psum",
                        )
                    for kt in range(KT):
                        for ot in range(g_start, g_end):
                            orows = min(P, O_SHARD_ - ot * P)
                            w_slice = w_slice_for(kt, g, ot - g_start, orows)
                            nc.tensor.matmul(
                                psums[ot][:orows, :], w_slice, rhs(kt),
                                start=(kt == 0), stop=(kt == KT - 1),
                            )
                    for ot in range(g_start, g_end):
                        evict(sc, ot, psums[ot])

            # Software-pipelined emission: loads for chunk sc+1 are emitted
            # before chunk sc's matmul groups, so in the per-queue FIFO
            # streams next-chunk loads sit ahead of this chunk's PSUM
            # drains.
            prev = rhs0
            for sc in range(N_CHUNKS):
                if sc + 1 < N_CHUNKS:
                    nxt = emit_x_chunk(sc + 1)
                else:
                    nxt = None
                emit_groups(sc, prev, tail=(sc == N_CHUNKS - 1))
                prev = nxt

    nc.compile()
    return nc


_NC_CACHE = None


def _get_nc():
    global _NC_CACHE
    if _NC_CACHE is None:
        _NC_CACHE = build_bass()
    return _NC_CACHE


def run(inputs, trace=False, trace_cores=None, tmpdir=None):
    x = np.asarray(inputs["x"])
    w = np.asarray(inputs["weight_int8"])
    scale = np.asarray(inputs["scale"], dtype=np.float32)
    bias = np.asarray(inputs["bias"], dtype=np.float32)

    if w.dtype != np.int8:
        w = w.astype(np.int8)
    x2d = np.ascontiguousarray(x.reshape(S, I).astype(np.float32, copy=False))
    xtr = np.ascontiguousarray(x2d.T.astype(np.float16))  # [I, S] fp16

    KT = I // P
    in_maps = []
    for c in range(N_CORES):
        sl = slice(c * O_SHARD, (c + 1) * O_SHARD)
        wtr = np.ascontiguousarray(w[sl, :].T)  # [I, O_SHARD]
        # wg0[p, kt*512+o] = wtr[kt*128+p, o] for o in [0, 512)
        wg0 = np.ascontiguousarray(
            wtr[:, :G0_W].reshape(KT, P, G0_W).transpose(1, 0, 2)
            .reshape(P, KT * G0_W).astype(np.float16))
        in_maps.append({
            "xt": xtr,
            "wg0": wg0,
            "wt": wtr,
            "scale": np.ascontiguousarray(scale[sl]),
            "bias": np.ascontiguousarray(bias[sl]),
        })

    nc = _get_nc()
    kwargs = {}
    if trace:
        kwargs["trace"] = True
        if trace_cores is not None:
            kwargs["trace_cores"] = trace_cores
        if tmpdir is not None:
            kwargs["tmpdir"] = tmpdir
    res = run_bass_kernel_spmd(nc, in_maps, core_ids=list(range(N_CORES)), **kwargs)

    yt_full = np.concatenate([res.results[c]["yt"] for c in range(N_CORES)], axis=0)
    out = np.ascontiguousarray(yt_full.T).reshape(B, S, O).astype(np.float32, copy=False)
    if trace:
        return out, res
    return out


def kernel(**inputs) -> np.ndarray:
    return run(inputs, trace=False)


# revision 13
# speedup vs baseline: 1.0869x; 1.0672x over previous
"""CompressedLinear (int8 weight, per-row scale) on 8 Trainium2 NeuronCores.

Math: y[b,s,o] = sum_i x[b,s,i] * (w_int8[o,i] * scale[o]) + bias[o]

Strategy (tensor-parallel over out_features, per sharding hint):
  - Shard W/scale/bias rows across 8 cores (1376 rows each); x replicated.
  - Scale is applied to the matmul OUTPUT (algebraically identical), so the
    device matmuls run on the raw int8 weights in fp16 (int8 is exact in
    fp16). A single fp16 pass with host-side fp16 x bounds the output
    relative error at ~2e-4.
  - Each core computes yT[o_shard, s] = W_shard @ x^T.
  - Load scheduling is latency-driven: a dma_start costs ~0.7-1us of queue
    issue time, in-flight descriptors round-robin across the DMA engines
    (a flood delays the first completion), and the gpsimd SWDGE *casting*
    path has ~6us fixed latency per block. So:
      * x chunk 0 rides the sync ring as multi-k-slice blocks with small
        head blocks, chained at depth 2 (low latency, full stream rate);
      * the first PSUM group's weights (o-columns 0:512) are pre-cast to
        fp16 on the host and ride the otherwise-idle scalar ring the same
        way, so the first (x, w) pair lands ~2us after the ~6.5us queue
        preamble and the PE never starves during the first group sweep;
      * groups 1/2 keep the wire-efficient int8->fp16 SWDGE cast stream
        (their deadlines are 30+us out), chained at depth 4.
  - Per-partition affine (scale, bias) is fused into the PSUM eviction.
  - The very last PSUM group runs kt-inner per o-tile, and the final
    o-tile is split into two s-halves, so the last evictions/output DMAs
    stagger into the matmul stream instead of serializing at the end.
"""

import os
import numpy as np

import concourse.bass as bass
import concourse.tile as tile
from concourse import bacc, mybir
from concourse.bass_utils import run_bass_kernel_spmd

B = 1
S = 2048
I = 4096
O = 11008
N_CORES = 8
O_SHARD = O // N_CORES  # 1376
S_CHUNK = 512
P = 128
KB = 8    # k-slices per x block, chunks 1+
HEAD_BLOCKS = [2, 2, 4, 4, 4, 4, 4, 4, 4]  # k-slices per block, chunk-0 x and g0 w
G0_W = 512  # o-columns in PSUM group 0 (4 o-tiles)


def build_bass(I_=I, O_SHARD_=O_SHARD, S_=S, S_CHUNK_=S_CHUNK):
    KT = I_ // P
    N_CHUNKS = S_ // S_CHUNK_
    OT = (O_SHARD_ + P - 1) // P

    MM_DT = mybir.dt.float16
    nc = bacc.Bacc("TRN2", target_bir_lowering=False, debug=False)

    xt = nc.dram_tensor("xt", [I_, S_], mybir.dt.float16, kind="ExternalInput").ap()
    # group-0 weights pre-cast to fp16 and pre-blocked on the host:
    # wg0[p, kt*512 + o] = W_shard^T[kt*128 + p, o] for o in [0, 512)
    wg0 = nc.dram_tensor("wg0", [P, KT * G0_W], mybir.dt.float16, kind="ExternalInput").ap()
    wt = nc.dram_tensor("wt", [I_, O_SHARD_], mybir.dt.int8, kind="ExternalInput").ap()
    scale = nc.dram_tensor("scale", [O_SHARD_], mybir.dt.float32, kind="ExternalInput").ap()
    bias = nc.dram_tensor("bias", [O_SHARD_], mybir.dt.float32, kind="ExternalInput").ap()
    yt = nc.dram_tensor("yt", [O_SHARD_, S_], mybir.dt.float32, kind="ExternalOutput").ap()

    # PSUM bank groups: 4+4+3 o-tiles so two adjacent groups fit in the
    # 8 banks and group transitions never wait on drains.
    groups = []
    g0 = 0
    for gsz in (4, 4, 3):
        if g0 < OT:
            groups.append((g0, min(g0 + gsz, OT)))
            g0 += gsz
    assert groups[0][1] * P == G0_W
    full_t = O_SHARD_ // P
    rem = O_SHARD_ - full_t * P

    with tile.TileContext(nc) as tc:
        with (
            tc.tile_pool(name="wres", bufs=1) as wres_pool,
            tc.tile_pool(name="consts", bufs=1) as const_pool,
            tc.tile_pool(name="xc0", bufs=1) as x0_pool,
            tc.tile_pool(name="xcn", bufs=2 * (KT // KB)) as xn_pool,
            tc.tile_pool(name="outp", bufs=4) as out_pool,
            tc.tile_pool(name="psum", bufs=8, space="PSUM") as psum_pool,
        ):
            # PE warm-up: dependency-free matmuls on a zeroed tile keep the
            # PE busy from right after the preamble, so the HAM clock gate
            # opens (K=8/8) around when the first real matmuls flow.
            warm_sb = const_pool.tile([P, P], MM_DT)
            nc.any.memset(warm_sb[:], 0.0)
            warm_ps = psum_pool.tile([P, P], mybir.dt.float32, name="warm_ps", tag="psum")
            N_WARM = 16
            for i in range(N_WARM):
                nc.tensor.matmul(
                    warm_ps[:], warm_sb[:], warm_sb[:],
                    start=(i == 0), stop=(i == N_WARM - 1),
                )

            def chain(dd, dds, depth, reason):
                if len(dds) >= depth:
                    bass._add_dep_helper(
                        dd.ins, dds[-depth].ins, sync=True, reason=reason)
                dds.append(dd)

            # All x blocks ride the sync ring in ONE continuous depth-2
            # chain: at most two descriptors in flight, so the urgent
            # chunk-0 head stays low-latency and later bulk chunks can
            # never dilute another stream's bandwidth share. (In-flight
            # descriptors round-robin across the DMA engines, so an
            # unchained flood starves whatever is urgent.)
            x_dds = []

            def emit_x_chunk0():
                blocks = []  # (kt0, kb, tile)
                kt0 = 0
                for i, kb in enumerate(HEAD_BLOCKS):
                    bt = x0_pool.tile([P, kb * S_CHUNK_], MM_DT, tag=f"x0b{i}")
                    src = xt[kt0 * P:(kt0 + kb) * P, 0:S_CHUNK_]
                    dd = nc.sync.dma_start(
                        bt[:].rearrange("p (kt s) -> p kt s", s=S_CHUNK_),
                        src.rearrange("(kt p) s -> p kt s", p=P))
                    chain(dd, x_dds, 2, "depth-2 chain: low-latency x head")
                    blocks.append((kt0, kb, bt))
                    kt0 += kb
                def rhs(kt, blocks=blocks):
                    for kt0, kb, bt in blocks:
                        if kt0 <= kt < kt0 + kb:
                            return bt[:, (kt - kt0) * S_CHUNK_:(kt - kt0 + 1) * S_CHUNK_]
                    raise KeyError(kt)
                return rhs

            def emit_x_chunk(sc):
                s0 = sc * S_CHUNK_
                blocks = []
                for b in range(KT // KB):
                    bt = xn_pool.tile([P, KB * S_CHUNK_], MM_DT, tag=f"xb{KB}")
                    src = xt[b * KB * P:(b + 1) * KB * P, s0:s0 + S_CHUNK_]
                    dd = nc.sync.dma_start(
                        bt[:].rearrange("p (kt s) -> p kt s", s=S_CHUNK_),
                        src.rearrange("(kt p) s -> p kt s", p=P))
                    chain(dd, x_dds, 2, "depth-2 chain: x bulk paced behind head")
                    blocks.append(bt)
                def rhs(kt, blocks=blocks):
                    return blocks[kt // KB][:, (kt % KB) * S_CHUNK_:(kt % KB + 1) * S_CHUNK_]
                return rhs

            rhs0 = emit_x_chunk0()

            # group-0 weights: host-pre-cast fp16 on the scalar ring,
            # same head-block/chaining scheme as x chunk 0. (The SWDGE
            # cast path has ~6us fixed latency - too slow for group 0.)
            wg0_blocks = []  # (kt0, kb, tile)
            wg0_dds = []
            kt0 = 0
            for i, kb in enumerate(HEAD_BLOCKS):
                w_b = wres_pool.tile([P, kb * G0_W], MM_DT, tag=f"wg0_{i}")
                dd = nc.scalar.dma_start(
                    w_b[:], wg0[:, kt0 * G0_W:(kt0 + kb) * G0_W])
                chain(dd, wg0_dds, 2, "depth-2 chain: low-latency w g0 head")
                wg0_blocks.append((kt0, kb, w_b))
                kt0 += kb

            # per-partition scale/bias columns, scalar ring after the g0
            # weights (tiny; first needed at the first eviction ~35us in).
            scale_t = const_pool.tile([P, OT], mybir.dt.float32)
            bias_t = const_pool.tile([P, OT], mybir.dt.float32)
            if full_t:
                nc.scalar.dma_start(
                    scale_t[:, :full_t], scale[: full_t * P].rearrange("(t p) -> p t", p=P)
                )
                nc.scalar.dma_start(
                    bias_t[:, :full_t], bias[: full_t * P].rearrange("(t p) -> p t", p=P)
                )
            if rem:
                nc.scalar.dma_start(
                    scale_t[:rem, full_t:], scale[full_t * P:].rearrange("(t p) -> p t", p=rem)
                )
                nc.scalar.dma_start(
                    bias_t[:rem, full_t:], bias[full_t * P:].rearrange("(t p) -> p t", p=rem)
                )

            # groups 1/2: int8 -> fp16 SWDGE cast stream on gpsimd (wire-
            # efficient; deadlines 30+us out), KWB k-slices per block,
            # chained at depth 4 to balance latency and aggregate rate.
            # The whole stream is held back until the group-0 weights have
            # landed so it cannot dilute the urgent startup streams'
            # bandwidth share (descriptor-level round-robin).
            KWB = 4
            w_blocks = [None] * len(groups)
            w_widths = [None] * len(groups)
            dds = [wg0_dds[-1]] * 4
            for g, (g_start, g_end) in enumerate(groups):
                if g == 0:
                    continue
                c0 = g_start * P
                c1 = min(g_end * P, O_SHARD_)
                wid = c1 - c0
                w_widths[g] = wid
                blocks = []
                for b in range(KT // KWB):
                    w_b = wres_pool.tile([P, KWB * wid], MM_DT, tag=f"wb{g}_{b}")
                    src = wt[b * KWB * P:(b + 1) * KWB * P, c0:c1]
                    dd = nc.gpsimd.dma_start(
                        w_b[:].rearrange("p (kt o) -> p kt o", o=wid),
                        src.rearrange("(kt p) o -> p kt o", p=P))
                    chain(dd, dds, 4, "depth-4 chain: pace SWDGE w cast stream")
                    blocks.append(w_b)
                w_blocks[g] = blocks

            def w_slice_for(kt, g, ot_local, orows):
                if g == 0:
                    for kt0, kb, w_b in wg0_blocks:
                        if kt0 <= kt < kt0 + kb:
                            base = (kt - kt0) * G0_W + ot_local * P
                            return w_b[:, base:base + orows]
                    raise KeyError(kt)
                wid = w_widths[g]
                base = (kt % KWB) * wid + ot_local * P
                return w_blocks[g][kt // KWB][:, base:base + orows]

            def evict(sc, ot, psum_t, s_off=0, s_len=None):
                s_len = S_CHUNK_ if s_len is None else s_len
                s0 = sc * S_CHUNK_ + s_off
                orows = min(P, O_SHARD_ - ot * P)
                out_t = out_pool.tile([P, S_CHUNK_], mybir.dt.float32)
                nc.vector.tensor_scalar(
                    out=out_t[:orows, :s_len],
                    in0=psum_t[:orows, :s_len],
                    scalar1=scale_t[:orows, ot:ot + 1],
                    scalar2=bias_t[:orows, ot:ot + 1],
                    op0=mybir.AluOpType.mult,
                    op1=mybir.AluOpType.add,
                )
                nc.sync.dma_start(
                    yt[ot * P:ot * P + orows, s0:s0 + s_len],
                    out_t[:orows, :s_len],
                )

            def emit_groups(sc, rhs, tail=False):
                # kt outer / o-tile inner: each x block's last reader comes
                # early in the group sweep, so next-chunk loads spread over
                # the whole chunk instead of bunching at its tail.
                for g, (g_start, g_end) in enumerate(groups):
                    last_group = tail and g == len(groups) - 1
                    if last_group:
                        # kt-inner per o-tile: each o-tile completes ~7us
                        # apart, so evictions/output DMAs overlap the
                        # remaining matmuls. The final o-tile is further
                        # split into two s-halves for the same reason.
                        for ot in range(g_start, g_end):
                            orows = min(P, O_SHARD_ - ot * P)
                            halves = ((0, S_CHUNK_),) if ot < g_end - 1 else (
                                (0, S_CHUNK_ // 2), (S_CHUNK_ // 2, S_CHUNK_ // 2))
                            for s_off, s_len in halves:
                                ps = psum_pool.tile(
                                    [P, s_len], mybir.dt.float32,
                                    name=f"psum_{sc}_{ot}_{s_off}", tag="psum",
                                )
                                for kt in range(KT):
                                    w_slice = w_slice_for(kt, g, ot - g_start, orows)
                                    nc.tensor.matmul(
                                        ps[:orows, :], w_slice,
                                        rhs(kt)[:, s_off:s_off + s_len],
                                        start=(kt == 0), stop=(kt == KT - 1),
                                    )
                                evict(sc, ot, ps, s_off, s_len)
                        continue
                    psums = {}
                    for ot in range(g_start, g_end):
                        psums[ot] = psum_pool.tile(
                            [P, S_CHUNK_], mybir.dt.float32,
                            name=f"psum_{sc}_{ot}", tag="psum",
                        )
                    for kt in range(KT):
                        for ot in range(g_start, g_end):
                            orows = min(P, O_SHARD_ - ot * P)
                            w_slice = w_slice_for(kt, g, ot - g_start, orows)
                            nc.tensor.matmul(
                                psums[ot][:orows, :], w_slice, rhs(kt),
                                start=(kt == 0), stop=(kt == KT - 1),
                            )
                    for ot in range(g_start, g_end):
                        evict(sc, ot, psums[ot])

            # Software-pipelined emission: loads for chunk sc+1 are emitted
            # before chunk sc's matmul groups, so in the per-queue FIFO
            # streams next-chunk loads sit ahead of this chunk's PSUM
            # drains.
            prev = rhs0
            for sc in range(N_CHUNKS):
                if sc + 1 < N_CHUNKS:
                    nxt = emit_x_chunk(sc + 1)
                else:
                    nxt = None
                emit_groups(sc, prev, tail=(sc == N_CHUNKS - 1))
                prev = nxt

    nc.compile()
    return nc


_NC_CACHE = None


def _get_nc():
    global _NC_CACHE
    if _NC_CACHE is None:
        _NC_CACHE = build_bass()
    return _NC_CACHE


def run(inputs, trace=False, trace_cores=None, tmpdir=None):
    x = np.asarray(inputs["x"])
    w = np.asarray(inputs["weight_int8"])
    scale = np.asarray(inputs["scale"], dtype=np.float32)
    bias = np.asarray(inputs["bias"], dtype=np.float32)

    if w.dtype != np.int8:
        w = w.astype(np.int8)
    x2d = np.ascontiguousarray(x.reshape(S, I).astype(np.float32, copy=False))
    xtr = np.ascontiguousarray(x2d.T.astype(np.float16))  # [I, S] fp16

    KT = I // P
    in_maps = []
    for c in range(N_CORES):
        sl = slice(c * O_SHARD, (c + 1) * O_SHARD)
        wtr = np.ascontiguousarray(w[sl, :].T)  # [I, O_SHARD]
        # wg0[p, kt*512+o] = wtr[kt*128+p, o] for o in [0, 512)
        wg0 = np.ascontiguousarray(
            wtr[:, :G0_W].reshape(KT, P, G0_W).transpose(1, 0, 2)
            .reshape(P, KT * G0_W).astype(np.float16))
        in_maps.append({
            "xt": xtr,
            "wg0": wg0,
            "wt": wtr,
            "scale": np.ascontiguousarray(scale[sl]),
            "bias": np.ascontiguousarray(bias[sl]),
        })

    nc = _get_nc()
    kwargs = {}
    if trace:
        kwargs["trace"] = True
        if trace_cores is not None:
            kwargs["trace_cores"] = trace_cores
        if tmpdir is not None:
            kwargs["tmpdir"] = tmpdir
    res = run_bass_kernel_spmd(nc, in_maps, core_ids=list(range(N_CORES)), **kwargs)

    yt_full = np.concatenate([res.results[c]["yt"] for c in range(N_CORES)], axis=0)
    out = np.ascontiguousarray(yt_full.T).reshape(B, S, O).astype(np.float32, copy=False)
    if trace:
        return out, res
    return out


def kernel(**inputs) -> np.ndarray:
    return run(inputs, trace=False)
